# revision 50
# baseline (speedup 1.0000x reference)
"""CORAL loss kernel for Trainium2 (8 NeuronCores, Bass/Tile).

Strategy (data-parallel over bz, per sharding hint):
  - Shard features [32, 4096, 256] along bz: 4 batch elements per core.
  - Host casts features to fp16 and appends a ones column (d -> d+1), so the
    device reads half the bytes and the PE runs single-pass matmuls (fp32
    matmuls lower to two LO/HI passes on TRN2 and are ~4x slower). PSUM
    accumulation stays fp32; the loss error from fp16 inputs is ~3e-6
    relative (the CORAL loss is a large average, so per-element quantization
    noise washes out; measured end to end).
  - Per batch element b on device: partition p of SBUF holds 32 consecutive
    rows of xaug[b] (any partition of the n rows is valid for sum_n x x^T,
    and consecutive rows give long contiguous DMA runs -> full HBM BW).
    The PE accumulates, in PSUM, ps0 = S rows 0:128 (all 257 cols: S block
    plus the colsum column from the ones trick) and ps1 = S rows 128:256,
    cols 128:257 only (S is symmetric; the host mirrors the lower block).
    DVE stages PSUM to SBUF as fp16 (halves the write traffic; adds only
    ~1e-6 loss error); one DMA per batch writes the packed block out.
  - Host (float64): reassemble S, cov_b = (S_b - colsum_b x m_b)/(n-1) with
    m_b = colsum_b/n, then the tiny masked pairwise CORAL reduction (exact
    mirror of the reference math). ~10 MFLOP on 6.3 MB of stats - gather
    work, like the all-gather + replicated reduction in the sharding hint.

Hardware notes:
  - Most instructions carry at most ONE semaphore wait (PE Matmult/
    Ldweights, DMA descriptors), so the structure keeps every instruction
    at <=1 wait: the ones column arrives with the data DMA (single producer
    per x tile), x tiles get dedicated SBUF slots (no reuse -> x DMAs never
    wait), PSUM banks are claimed by a tiny const-only matmul pinned
    (order-only dep) after the bank's previous user's PE "fence", and the
    fence reads the staged output tile so the DVE-release of the bank is
    transitively implied. Out-DMAs go through SWDGE (gpsimd) so they stay
    off the FIFO HWDGE ring that feeds the x loads. Tile's kernel-tail
    Drain is split into single-wait drains by a JSON post-pass (the HW
    allows one sync wait per instruction).
  - The PE clock is HAM-gated (1.2 GHz until ~3.4 us of sustained
    activity): a burst of warm-up matmuls on a memset constant runs during
    the framework preamble so the real matmul stream starts at 2.4 GHz.
"""

import sys

import numpy as np

if "/opt/trn_rl_repo" not in sys.path:
    sys.path.insert(0, "/opt/trn_rl_repo")

import concourse.bass as bass
import concourse.mybir as mybir
import concourse.tile as tile
from concourse.tile_rust import add_dep_helper

BZ, N, D = 32, 4096, 256
NCORES = 8
BPC = BZ // NCORES  # batch elements per core
P = 128  # partitions


def build_nc(bpc=BPC, n=N, d=D, kc=8, ps_bufs=3, warmup=16, warmn=512, xp_bufs=None):
    """Per-core Bass module: raw S blocks for `bpc` batch elements.

    Input "x": host-prepared fp16 [bpc, n, d+1] ([X | ones]).
    Output "outs": fp16 [bpc, 128, 386] packed per-batch blocks
    [S[0:128, 0:256] | colsum[0:128]] ++ [S[128:256, 128:256] | colsum[128:256]].
    """
    assert n % P == 0 and d == 2 * P
    kt = n // P  # k-tiles of 128 rows
    assert kt % kc == 0
    nchunk = kt // kc  # DMA chunks per batch element
    if xp_bufs is None:
        # One slot per chunk-load: x-tile slots are never reused, so x DMAs
        # never need a slot-release wait (DMAs also carry at most one wait).
        xp_bufs = bpc * nchunk

    nc = bass.Bass(trn_type="TRN2", enable_partition_id=False)
    f32 = mybir.dt.float32
    f16 = mybir.dt.float16
    x = nc.dram_tensor("x", [bpc, n, d + 1], f16, kind="ExternalInput")
    w0, w1 = d + 1, d // 2 + 1
    # fp16 stats output: halves the write traffic; S diag ~n gives fp16 abs
    # err ~2 -> cov err ~5e-4 per diag entry, which averages out to ~1e-5
    # relative on the loss (verified against the fp32-output variant).
    outs = nc.dram_tensor("outs", [bpc, P, w0 + w1], f16, kind="ExternalOutput")

    with tile.TileContext(nc) as tc:
        with (
            tc.tile_pool(name="xp", bufs=xp_bufs) as xp,
            tc.tile_pool(name="op", bufs=bpc) as op,
            tc.tile_pool(name="constp", bufs=1) as constp,
            tc.tile_pool(name="psp", bufs=ps_bufs, space="PSUM") as psp,
            tc.tile_pool(name="warmp", bufs=1, space="PSUM") as warmp,
        ):
            # Constant operand for warm-up/claim matmuls (DVE memset: cheap,
            # runs during the framework preamble).
            wrm = constp.tile([P, warmn], f16)
            nc.vector.memset(wrm[:, :], 1.0)

            # HAM warm-up: keep the PE busy through the preamble so the
            # clock gate is at 8/8 (2.4 GHz) when the real stream starts.
            wps = warmp.tile([1, warmn], f32)
            for _ in range(warmup):
                nc.tensor.matmul(
                    wps[0:1, :], wrm[:, 0:1], wrm[:, 0:warmn],
                    start=True, stop=True, skip_group_check=True,
                )

            def claim(pstile, after=None):
                # Tiny const-only matmul whose only job is to carry the PSUM
                # bank slot-release wait (one-wait-per-PE-instruction limit).
                # Garbage value; cleared by start=True of the first real use.
                inst = nc.tensor.matmul(
                    pstile[0:1, 0:1], wrm[:, 0:1], wrm[:, 0:1],
                    start=True, stop=True, skip_group_check=True,
                )
                if after is not None:
                    # Pin the claim after the fence of the bank's previous
                    # user (same engine, order-only): the DVE-release wait is
                    # then implied by the fence's wait and elided, leaving
                    # only the PE bank-drain wait.
                    add_dep_helper(inst.ins, after.ins, sync=False,
                                   reason="psum claim after fence")
                return inst

            # Per-batch chunk splits (in k-tiles). Batch 0 leads with tiny
            # chunks: the first DMA completes almost alone at full bandwidth,
            # so the PE starts ~5us earlier than with uniform chunks (many
            # concurrent in-flight DMAs share HBM fairly, delaying the first
            # completion).
            def chunk_split(b):
                return [kc] * (kt // kc)

            # Issue ALL x loads up front: each gets a dedicated SBUF slot and
            # has no dependencies, and the Sync HWDGE ring is FIFO - a store
            # emitted between loads would block later loads behind its wait.
            xts = {}
            for b in range(bpc):
                k0 = 0
                for c, kcc in enumerate(chunk_split(b)):
                    xt = xp.tile([P, kcc, d + 1], f16, tag=f"xt{kcc}",
                                 name=f"xt_{b}_{c}")
                    # Partition p holds consecutive rows -> contiguous DMA.
                    src = x[b].rearrange("(p k) e -> p k e", p=P)[
                        :, k0 : k0 + kcc, :
                    ]
                    nc.sync.dma_start(out=xt[:, :, :], in_=src)
                    xts[b, c] = xt
                    k0 += kcc

            def emit_kloop(b, fence=None):
                ps0 = psp.tile([P, w0], f32, tag="ps0", name=f"ps0_{b}")
                ps1 = psp.tile([P, w1], f32, tag="ps1", name=f"ps1_{b}")
                claim(ps0, after=fence)
                claim(ps1, after=fence)
                kk = 0
                for c, kcc in enumerate(chunk_split(b)):
                    xt = xts[b, c]
                    for k in range(kcc):
                        nc.tensor.matmul(
                            ps0[:, :], xt[:, k, 0:P], xt[:, k, :],
                            start=(kk == 0), stop=(kk == kt - 1),
                        )
                        nc.tensor.matmul(
                            ps1[:, :], xt[:, k, P:d], xt[:, k, P : d + 1],
                            start=(kk == 0), stop=(kk == kt - 1),
                        )
                        kk += 1
                return ps0, ps1

            def emit_epilogue(b, ps0, ps1):
                ot = op.tile([P, w0 + w1], f16, tag="ot", name=f"ot_{b}")
                nc.vector.tensor_copy(ot[:, 0:w0], ps0[:, :])
                nc.vector.tensor_copy(ot[:, w0 : w0 + w1], ps1[:, :])
                # Out-DMAs via SWDGE (gpsimd): the x loads use all 8 HWDGE
                # semaphore lanes, and the Sync HWDGE ring is FIFO (a store's
                # wait would block later loads queued behind it).
                nc.gpsimd.dma_start(out=outs[b], in_=ot[:, :])
                # PE fence: reads the region written by the LAST DVE copy,
                # so the PE's observed DVE clock passes both PSUM reads; the
                # next claim of these banks then needs no explicit DVE wait.
                # Writes garbage into ps0 after its data was staged.
                return nc.tensor.matmul(
                    ps0[0:1, 0:1],
                    ot[:, w0 + w1 - 1 : w0 + w1], ot[:, w0 + w1 - 1 : w0 + w1],
                    start=True, stop=True, skip_group_check=True,
                )

            # One-batch software pipeline: epilogue(b) is emitted after
            # kloop(b+1) so the PE stream never stalls on the epilogue.
            prev = None
            fences = {}
            for b in range(bpc):
                cur = emit_kloop(b, fence=fences.get(b - ps_bufs))
                if prev is not None:
                    fences[b - 1] = emit_epilogue(b - 1, *prev)
                prev = cur
            emit_epilogue(bpc - 1, *prev)

    _install_drain_split(nc)
    return nc


def _split_drain_waits(bir, max_waits=1):
    """Split any Drain carrying more than `max_waits` sem waits into a chain
    of single-wait Drains (the HW sync-wait table is tiny; Tile's kernel-tail
    drain waits on every active sem lane at once)."""
    for fn in bir["functions"]:
        for blk in fn["blocks"]:
            out = []
            changed = False
            for inst in blk["instructions"]:
                waits = (inst.get("sync_info") or {}).get("on_wait") or []
                if inst.get("opcode") == "Drain" and len(waits) > max_waits:
                    changed = True
                    for wi in range(0, len(waits) - max_waits):
                        clone = {
                            **inst,
                            "name": f"{inst['name']}_w{wi}",
                            "sync_info": {
                                "on_wait": [waits[wi]],
                                "on_update": [],
                            },
                        }
                        out.append(clone)
                    inst = {
                        **inst,
                        "sync_info": {
                            **inst["sync_info"],
                            "on_wait": waits[len(waits) - max_waits :],
                        },
                    }
                out.append(inst)
            if changed:
                blk["instructions"] = out
    return bir


def _install_drain_split(nc):
    import orjson

    raw = nc.to_json_bytes

    def patched():
        return orjson.dumps(_split_drain_waits(orjson.loads(raw())))

    nc.to_json_bytes = patched


_NC_CACHE = {}


def _get_nc():
    key = (BPC, N, D)
    if key not in _NC_CACHE:
        _NC_CACHE[key] = build_nc()
    return _NC_CACHE[key]


def augment_ones_f16(feats, bpc, n, d):
    """[cores, bpc, n, d] fp32 -> per-core fp16 [bpc, n, d+1] with ones."""
    out = np.empty((feats.shape[0], bpc, n, d + 1), dtype=np.float16)
    out[..., :d] = feats
    out[..., d] = 1.0
    return out


def stats_from_raw(outs_blocks, n=N, d=D):
    """Device outs [bz, 128, 386] (packed, see build_nc) -> f64 stats."""
    bz = outs_blocks.shape[0]
    h = d // 2
    o = outs_blocks.astype(np.float64)
    s = np.empty((bz, d, d))
    s[:, :h, :] = o[:, :, 0:d]
    s[:, h:, h:] = o[:, :, d + 1 : d + 1 + h]
    s[:, h:, :h] = np.swapaxes(o[:, :, h:d], 1, 2)  # symmetry mirror
    colsum = np.concatenate([o[:, :, d], o[:, :, d + 1 + h]], axis=1)
    m = colsum / n
    covs = (s - colsum[:, :, None] * m[:, None, :]) / (n - 1)
    return m, covs


def coral_from_stats(means, covs, domains, d=D):
    """Masked pairwise CORAL reduction from per-batch stats (float64)."""
    bz = means.shape[0]
    m = means.astype(np.float64)
    ms = (m * m).sum(1)
    md = (ms[:, None] + ms[None, :] - 2.0 * (m @ m.T)) / d
    v = covs.astype(np.float64).reshape(bz, -1)
    cs = (v * v).sum(1)
    g = v @ v.T
    cd = (cs[:, None] + cs[None, :] - 2.0 * g) / (d * d)
    upper = np.triu(np.ones((bz, bz), dtype=bool), k=1)
    mask = upper & (np.asarray(domains)[:, None] != np.asarray(domains)[None, :])
    loss = np.where(mask, md + cd, 0.0).sum()
    num = int(mask.sum())
    if num > 1:
        loss = loss / num
    return np.float32(loss)


def kernel(features, domains, _trace=False):
    from concourse import bass_utils

    feats = np.asarray(features)
    assert feats.shape == (BZ, N, D)
    xaug = augment_ones_f16(
        np.asarray(feats, dtype=np.float32).reshape(NCORES, BPC, N, D), BPC, N, D
    )
    nc = _get_nc()
    in_maps = [{"x": xaug[c]} for c in range(NCORES)]
    res = bass_utils.run_bass_kernel_spmd(
        nc, in_maps, core_ids=list(range(NCORES)), trace=_trace
    )
    blocks = np.concatenate([r["outs"] for r in res.results], axis=0)
    means, covs = stats_from_raw(blocks)
    out = coral_from_stats(means, covs, domains)
    if _trace:
        return out, res
    return out
